# revision 4
# baseline (speedup 1.0000x reference)
"""Trainium2 Bass kernel for the KinematicBicycle rollout (H=8192).

kernel(x0, U, dt) -> [8193, 4] float32 trajectory, computed on TRN2.

Algorithm (validated against the jax reference):
  b_t = clip(U[t,0],±3)*dt. The speed recurrence
      w_{t+1} = min(max(w_t + b_t, 0), 30)
  is an alternating clamp chain, computed EXACTLY with the DVE
  tensor_tensor_scan primitive (state = (data0 op0 state) op1 data1) using
  op0=subtract / op1=max over sign-alternating interleaved data:
      even step: state = max(b_t - state, 0)    # state_in = -w_t
      odd  step: state = max(0 - state, -30)    # = -min(., 30) = -w_{t+1}
  Time is chunked [128 partitions x 64 steps]. Chunk-entry speeds come from
  composing per-chunk clamp-add maps F_p(v)=min(max(v+s_p,lo_p),hi_p):
  lo_p/hi_p are obtained by probing each chunk scan with -/+BIG initials, and
  a 4-micro-step composition scan (512 elements on one partition) chains the
  128 maps. Given w, theta is a plain prefix sum of w_t*tan(delta_t)*dt/L and
  x/y are prefix sums of w_t*cos/sin(theta_t)*dt: per-partition
  tensor_tensor_scan + strict-lower-triangular matmul for cross-chunk
  offsets. ACT has only Sin: cos(x)=sin(x+pi/2); tan(d)=sin(d)/cos(d).

The rollout is a single sequential recurrence (no batch dim), so there is
nothing to shard across cores; the program is replicated SPMD on all 8
cores and core 0's output is returned.
"""
import os
import numpy as np

import concourse.bacc as bacc
import concourse.mybir as mybir
import concourse.tile as tile
from concourse.bass_utils import run_bass_kernel_spmd

F32 = mybir.dt.float32
OP = mybir.AluOpType
AF = mybir.ActivationFunctionType

H, P, C = 8192, 128, 64
L = 2.7
BIG = 1e30
HPI = float(np.pi / 2)
N_CORES = 8

LAST_RUN_INFO = {}
_CACHE = {}


def _host_constants():
    k = np.arange(P)[:, None]
    m = np.arange(P)[None, :]
    tri = (k < m).astype(np.float32)          # lhsT: out[m] = sum_{k<m} rhs[k]
    eye = (k == m).astype(np.float32)
    return tri, eye


def _build():
    nc = bacc.Bacc("TRN2", target_bir_lowering=False, debug=False)

    x0_d = nc.dram_tensor("x0", [4], F32, kind="ExternalInput")
    U_d = nc.dram_tensor("U", [H, 2], F32, kind="ExternalInput")
    dt_d = nc.dram_tensor("dt", [1], F32, kind="ExternalInput")
    tri_d = nc.dram_tensor("tri", [P, P], F32, kind="ExternalInput")
    eye_d = nc.dram_tensor("eye", [P, P], F32, kind="ExternalInput")
    out_d = nc.dram_tensor("out", [H + 1, 4], F32, kind="ExternalOutput")

    with tile.TileContext(nc) as tc:
        with (
            tc.tile_pool(name="sb", bufs=1) as sb,
            tc.tile_pool(name="ps", bufs=1, space="PSUM") as ps,
        ):
            # ---- loads ----
            Ut = sb.tile([P, 2 * C], F32, tag="Ut")
            nc.sync.dma_start(out=Ut, in_=U_d[:].rearrange("(p j) c -> p (j c)", p=P))
            xrow = sb.tile([1, 8], F32, tag="xrow")
            nc.sync.dma_start(out=xrow[0:1, 0:4],
                              in_=x0_d[:].rearrange("(o a) -> o a", o=1))
            nc.sync.dma_start(out=xrow[0:1, 4:5],
                              in_=dt_d[:].rearrange("(o a) -> o a", o=1))
            tri_t = sb.tile([P, P], F32, tag="tri")
            nc.sync.dma_start(out=tri_t, in_=tri_d[:, :])
            eye_t = sb.tile([P, P], F32, tag="eye")
            nc.sync.dma_start(out=eye_t, in_=eye_d[:, :])

            # ---- scalar prep on partition 0: dt/L, e0=clip(x0v), -e0 ----
            nc.vector.tensor_scalar_mul(xrow[0:1, 5:6], xrow[0:1, 4:5], 1.0 / L)
            nc.vector.tensor_scalar(xrow[0:1, 6:7], xrow[0:1, 3:4],
                                    0.0, 30.0, OP.max, OP.min)
            nc.vector.tensor_scalar_mul(xrow[0:1, 7:8], xrow[0:1, 6:7], -1.0)

            # ---- broadcast x0/dt-derived scalars to all partitions ----
            ones_row = sb.tile([1, P], F32, tag="ones_row")
            nc.vector.memset(ones_row, 1.0)
            one_t = sb.tile([1, 1], F32, tag="one_t")
            nc.vector.memset(one_t, 1.0)
            xb_ps = ps.tile([P, 8], F32, tag="xb")
            nc.tensor.matmul(xb_ps, ones_row, xrow, start=True, stop=True)
            xbs = sb.tile([P, 8], F32, tag="xbs")
            nc.vector.tensor_copy(xbs, xb_ps)
            x00, y00, th0 = xbs[:, 0:1], xbs[:, 1:2], xbs[:, 2:3]
            dt_b, dtL_b = xbs[:, 4:5], xbs[:, 5:6]

            # ---- controls: b = clip(a)*dt ; tanl = tan(clip(delta))*dt/L ----
            bcl = sb.tile([P, C], F32, tag="bcl")
            nc.vector.tensor_scalar(bcl, Ut[:, 0:2 * C:2], -3.0, 3.0, OP.max, OP.min)
            b = sb.tile([P, C], F32, tag="b")
            nc.vector.tensor_scalar_mul(b, bcl, dt_b)
            dcl = sb.tile([P, C], F32, tag="dcl")
            nc.vector.tensor_scalar(dcl, Ut[:, 1:2 * C:2], -0.6, 0.6, OP.max, OP.min)
            hpi_b = sb.tile([P, 1], F32, tag="hpi_b")
            nc.vector.memset(hpi_b, HPI)
            zero_b = sb.tile([P, 1], F32, tag="zero_b")
            nc.vector.memset(zero_b, 0.0)
            sin_d = sb.tile([P, C], F32, tag="sin_d")
            nc.scalar.activation(sin_d, dcl, AF.Sin, bias=zero_b)
            cos_d = sb.tile([P, C], F32, tag="cos_d")
            nc.scalar.activation(cos_d, dcl, AF.Sin, bias=hpi_b)
            rcos = sb.tile([P, C], F32, tag="rcos")
            nc.vector.reciprocal(rcos, cos_d)
            tanl = sb.tile([P, C], F32, tag="tanl")
            nc.vector.tensor_tensor(tanl, sin_d, rcos, OP.mult)
            nc.vector.tensor_scalar_mul(tanl, tanl, dtL_b)

            # ---- v-scan pass 1: per-chunk probes ----
            d0v = sb.tile([P, 2 * C], F32, tag="d0v")
            nc.vector.memset(d0v, 0.0)
            nc.vector.tensor_copy(d0v[:, 0:2 * C:2], b)
            d1v = sb.tile([P, 2 * C], F32, tag="d1v")
            nc.vector.memset(d1v, 0.0)
            nc.vector.memset(d1v[:, 1:2 * C:2], -30.0)
            slo = sb.tile([P, 2 * C], F32, tag="slo")
            nc.vector.tensor_tensor_scan(slo, d0v, d1v, BIG, OP.subtract, OP.max)
            shi = sb.tile([P, 2 * C], F32, tag="shi")
            nc.vector.tensor_tensor_scan(shi, d0v, d1v, -BIG, OP.subtract, OP.max)
            s_col = sb.tile([P, 1], F32, tag="s_col")
            nc.vector.tensor_reduce(s_col, b, mybir.AxisListType.X, OP.add)

            # ---- transpose chunk maps to rows; compose scan for entries ----
            s_row = ps.tile([1, P], F32, tag="s_row")
            nc.tensor.matmul(s_row, s_col, eye_t, start=True, stop=True)
            hi_row = ps.tile([1, P], F32, tag="hi_row")   # holds -hi_p
            nc.tensor.matmul(hi_row, shi[:, 2 * C - 1:2 * C], eye_t,
                             start=True, stop=True)
            lo_row = ps.tile([1, P], F32, tag="lo_row")   # holds -lo_p
            nc.tensor.matmul(lo_row, slo[:, 2 * C - 1:2 * C], eye_t,
                             start=True, stop=True)
            d0c = sb.tile([1, 4 * P], F32, tag="d0c")
            nc.vector.memset(d0c, 0.0)
            nc.vector.tensor_copy(d0c[0:1, 0:4 * P:4], s_row)
            d1c = sb.tile([1, 4 * P], F32, tag="d1c")
            nc.vector.memset(d1c, -BIG)
            nc.vector.tensor_copy(d1c[0:1, 1:4 * P:4], hi_row)
            nc.scalar.activation(d1c[0:1, 2:4 * P:4], lo_row, AF.Copy, scale=-1.0)
            comp = sb.tile([1, 4 * P], F32, tag="comp")
            nc.vector.tensor_tensor_scan(comp, d0c, d1c, xrow[0:1, 7:8],
                                         OP.subtract, OP.max)
            neg_e_row = sb.tile([1, P], F32, tag="neg_e_row")
            nc.vector.tensor_copy(neg_e_row[0:1, 0:1], xrow[0:1, 7:8])
            nc.vector.tensor_copy(neg_e_row[0:1, 1:P], comp[0:1, 3:4 * P - 4:4])
            nec = ps.tile([P, 1], F32, tag="nec")        # -e_p per partition
            nc.tensor.matmul(nec, neg_e_row, one_t, start=True, stop=True)

            # ---- v-scan pass 2 + w_in/w_out ----
            sv = sb.tile([P, 2 * C], F32, tag="sv")
            nc.vector.tensor_tensor_scan(sv, d0v, d1v, nec[:, 0:1],
                                         OP.subtract, OP.max)
            OUT = sb.tile([P, 4 * C], F32, tag="OUT")
            nc.scalar.activation(OUT[:, 3:4 * C:4], sv[:, 1:2 * C:2],
                                 AF.Copy, scale=-1.0)   # w_{t+1}
            w_in = sb.tile([P, C], F32, tag="w_in")
            nc.scalar.activation(w_in[:, 1:C], sv[:, 1:2 * C - 2:2],
                                 AF.Copy, scale=-1.0)
            nc.scalar.activation(w_in[:, 0:1], nec, AF.Copy, scale=-1.0)

            # ---- theta ----
            zer = sb.tile([P, C], F32, tag="zer")
            nc.vector.memset(zer, 0.0)
            g = sb.tile([P, C], F32, tag="g")
            nc.vector.tensor_tensor(g, w_in, tanl, OP.mult)
            sg = sb.tile([P, C], F32, tag="sg")
            nc.vector.tensor_tensor_scan(sg, g, zer, 0.0, OP.add, OP.add)
            offg = ps.tile([P, 1], F32, tag="offg")
            nc.tensor.matmul(offg, tri_t, sg[:, C - 1:C], start=True, stop=True)
            offg_sb = sb.tile([P, 1], F32, tag="offg_sb")
            nc.vector.tensor_copy(offg_sb, offg)
            nc.vector.tensor_scalar(OUT[:, 2:4 * C:4], sg, offg_sb, th0,
                                    OP.add, OP.add)     # theta_{t+1}
            texc = sb.tile([P, C], F32, tag="texc")
            nc.vector.tensor_tensor(texc, sg, g, OP.subtract)
            th_in = sb.tile([P, C], F32, tag="th_in")
            nc.vector.tensor_scalar(th_in, texc, offg_sb, th0, OP.add, OP.add)

            # ---- positions ----
            # ACT Sin is only accurate on ~[-pi, pi]; range-reduce theta.
            # Round-to-nearest via the fp32 magic constant 1.5*2^23.
            MAGIC = 12582912.0
            INV2PI = float(1.0 / (2.0 * np.pi))
            TWOPI = float(2.0 * np.pi)
            q1 = sb.tile([P, C], F32, tag="q1")
            nc.vector.tensor_scalar(q1, th_in, INV2PI, MAGIC, OP.mult, OP.add)
            n1 = sb.tile([P, C], F32, tag="n1")
            nc.vector.tensor_scalar(n1, q1, MAGIC, TWOPI, OP.subtract, OP.mult)
            thr = sb.tile([P, C], F32, tag="thr")     # theta mod 2pi in [-pi,pi]
            nc.vector.tensor_tensor(thr, th_in, n1, OP.subtract)
            # cos arg: reduce phi = theta + pi/2 so it also lands in [-pi,pi]
            phi = sb.tile([P, C], F32, tag="phi")
            nc.vector.tensor_scalar_add(phi, th_in, HPI)
            q2 = sb.tile([P, C], F32, tag="q2")
            nc.vector.tensor_scalar(q2, phi, INV2PI, MAGIC, OP.mult, OP.add)
            n2 = sb.tile([P, C], F32, tag="n2")
            nc.vector.tensor_scalar(n2, q2, MAGIC, TWOPI, OP.subtract, OP.mult)
            thr2 = sb.tile([P, C], F32, tag="thr2")   # phi mod 2pi in [-pi,pi]
            nc.vector.tensor_tensor(thr2, phi, n2, OP.subtract)
            cos_t = sb.tile([P, C], F32, tag="cos_t")
            nc.scalar.activation(cos_t, thr2, AF.Sin, bias=zero_b)
            sin_t = sb.tile([P, C], F32, tag="sin_t")
            nc.scalar.activation(sin_t, thr, AF.Sin, bias=zero_b)
            w_dt = sb.tile([P, C], F32, tag="w_dt")
            nc.vector.tensor_scalar_mul(w_dt, w_in, dt_b)
            c = sb.tile([P, C], F32, tag="c")
            nc.vector.tensor_tensor(c, w_dt, cos_t, OP.mult)
            d = sb.tile([P, C], F32, tag="d")
            nc.vector.tensor_tensor(d, w_dt, sin_t, OP.mult)
            scn = sb.tile([P, C], F32, tag="scn")
            nc.vector.tensor_tensor_scan(scn, c, zer, 0.0, OP.add, OP.add)
            sdn = sb.tile([P, C], F32, tag="sdn")
            nc.vector.tensor_tensor_scan(sdn, d, zer, 0.0, OP.add, OP.add)
            offc = ps.tile([P, 1], F32, tag="offc")
            nc.tensor.matmul(offc, tri_t, scn[:, C - 1:C], start=True, stop=True)
            offd = ps.tile([P, 1], F32, tag="offd")
            nc.tensor.matmul(offd, tri_t, sdn[:, C - 1:C], start=True, stop=True)
            offc_sb = sb.tile([P, 1], F32, tag="offc_sb")
            nc.vector.tensor_copy(offc_sb, offc)
            offd_sb = sb.tile([P, 1], F32, tag="offd_sb")
            nc.vector.tensor_copy(offd_sb, offd)
            nc.vector.tensor_scalar(OUT[:, 0:4 * C:4], scn, offc_sb, x00,
                                    OP.add, OP.add)
            nc.vector.tensor_scalar(OUT[:, 1:4 * C:4], sdn, offd_sb, y00,
                                    OP.add, OP.add)

            # ---- stores ----
            nc.sync.dma_start(
                out=out_d[1:H + 1, :].rearrange("(p j) c -> p (j c)", p=P),
                in_=OUT)
            nc.sync.dma_start(out=out_d[0:1, 0:4], in_=xrow[0:1, 0:4])

    nc.compile()
    return nc


def kernel(x0, U, dt):
    if "nc" not in _CACHE:
        _CACHE["nc"] = _build()
    nc = _CACHE["nc"]

    tri, eye = _host_constants()
    in_map = {
        "x0": np.ascontiguousarray(np.asarray(x0, np.float32)),
        "U": np.ascontiguousarray(np.asarray(U, np.float32)),
        "dt": np.asarray(dt, np.float32).reshape(1),
        "tri": tri,
        "eye": eye,
    }
    in_maps = [in_map for _ in range(N_CORES)]

    trace = os.environ.get("KB_TRACE", "0") == "1"
    res = run_bass_kernel_spmd(nc, in_maps, list(range(N_CORES)), trace=trace)

    LAST_RUN_INFO.clear()
    LAST_RUN_INFO["exec_time_ns"] = res.exec_time_ns
    if res.instructions_and_trace is not None:
        LAST_RUN_INFO["trace_path"] = res.instructions_and_trace[1]

    return np.asarray(res.results[0]["out"], np.float32).reshape(H + 1, 4)
